# revision 31
# baseline (speedup 1.0000x reference)
"""Cross-attention Trainium2 kernel for nn_CrossAttention_37495064494692.

B=8 batches sharded 1/core across 8 NeuronCores (data parallel).
Per core: full cross-attention for one batch element, computed in
feature-on-partitions ("transposed") layouts so no on-device transposes
are needed. Matmuls run in bf16 (fp32 PSUM accumulation); biases and
softmax math stay fp32.

The text projection is folded on the host: Wtk = Wt@Wk, Wtv = Wt@Wv
(fp32, then bf16), btk = bt@Wk + bk, btv = bt@Wv + bv, so K and V come
straight from guide_vector with one device matmul each.

  KT  = Wtk^T @ guideT + btk      [E, L]   (head h = rows h*64..h*64+64)
  V   = guideT^T @ Wtv + btv      [L, E]   stored padded with a ones
                                           column per head (v_aug) so the
                                           PV matmul also produces the
                                           softmax denominator row.
  QT  = Wq^T @ queryT + bq        [E, S]   (streamed per 512-col s-chunk)
  sT  = kT_h^T(L-tile) @ qT_h     [L, S]   K=64; the u=0/u=1 matmuls of a
                                           head pair write the two halves
                                           of one 2-bank PSUM tile from PE
                                           row groups 0/64 (concurrent)
  aT  = exp(SCALE*sT + maskbias)  [L, S]   one wide ACT op per pair
  OT_h = [v_h | 1]^T @ aT         [65, S]  row 64 = sum_l aT = denom
  OT   = OT_h / denom             denom rows staged to 32-aligned SBUF
                                  slots, DMA-gathered to [16,512], one
                                  reciprocal, DRAM-bounce stride-0
                                  partition broadcasts, elementwise mul
  out  = OT^T @ Wo + bo           [S, E]   natural layout, DMA'd out

The whole normalize chain for chunk c and its Wo projection run during
chunk c+1's attention (interleaved per head pair), and QT of chunk c+1
is computed during chunk c, so the PE stays warm across chunk
boundaries while the Scalar engine drains the exp() work.
"""
import sys

sys.path.insert(0, "/opt/trn_rl_repo")

import ml_dtypes
import numpy as np

import concourse.bacc as bacc
import concourse.bass as bass
import concourse.tile as tile
from concourse import mybir
from concourse.bass_utils import run_bass_kernel_spmd

F32 = mybir.dt.float32
BF16 = mybir.dt.bfloat16
MMDT = BF16                      # dtype of all matmul operands
NPDT = ml_dtypes.bfloat16        # matching numpy dtype for host inputs

B, S, L = 8, 2048, 512
E, TE, H = 16 * 64, 768, 16
D = E // H
SCALE = D ** -0.5

SC = 512              # s-chunk width
N_SC = S // SC        # 4 s-chunks
N_E = E // 128        # 8 E-chunks
N_TE = TE // 128      # 6 TE-chunks
N_LT = L // 128       # 4 L-tiles
HP = H // 2           # 8 head pairs

TRACE = False
_CACHED_NC = None

Exp = mybir.ActivationFunctionType.Exp
Ident = mybir.ActivationFunctionType.Identity


def build_nc():
    nc = bacc.Bacc()

    queryT = nc.declare_dram_parameter("queryT", [E, S], MMDT, isOutput=False)
    guideT = nc.declare_dram_parameter("guideT", [TE, L], MMDT, isOutput=False)
    Wtk = nc.declare_dram_parameter("Wtk", [TE, E], MMDT, isOutput=False)
    Wtv = nc.declare_dram_parameter("Wtv", [TE, E], MMDT, isOutput=False)
    Wq = nc.declare_dram_parameter("Wq", [E, E], MMDT, isOutput=False)
    Wo = nc.declare_dram_parameter("Wo", [E, E], MMDT, isOutput=False)
    bq = nc.declare_dram_parameter("bq", [E], F32, isOutput=False)
    btk = nc.declare_dram_parameter("btk", [E], F32, isOutput=False)
    btv_r = nc.declare_dram_parameter("btv_r", [E], MMDT, isOutput=False)
    bo = nc.declare_dram_parameter("bo", [E], F32, isOutput=False)
    mbias = nc.declare_dram_parameter("mbias", [L], F32, isOutput=False)
    out = nc.declare_dram_parameter("out", [S, E], F32, isOutput=True)

    with tile.TileContext(nc) as tc:
        with (
            tc.tile_pool(name="res", bufs=1) as res,
            tc.tile_pool(name="psV", bufs=2, space="PSUM") as psV,
            tc.tile_pool(name="psA", bufs=1, space="PSUM") as psA,
            tc.tile_pool(name="psC", bufs=1, space="PSUM") as psC,
        ):
            # ---- resident small tensors ----
            bq_sb = res.tile([128, N_E], F32, tag="bq")
            btk_sb = res.tile([128, N_E], F32, tag="btk")
            mb_sb = res.tile([128, N_LT], F32, tag="mb")
            nc.sync.dma_start(out=bq_sb, in_=bq.rearrange("(t p) -> p t", p=128))
            nc.sync.dma_start(out=btk_sb, in_=btk.rearrange("(t p) -> p t", p=128))
            nc.sync.dma_start(out=mb_sb, in_=mbias.rearrange("(t p) -> p t", p=128))
            bo_bc = res.tile([128, E], F32, tag="bo")
            bo_ap = bo[:]
            nc.gpsimd.dma_start(
                out=bo_bc,
                in_=bass.AP(tensor=bo_ap.tensor, offset=bo_ap.offset,
                            ap=[[0, 128], [1, E]]),
            )
            bv_row = res.tile([1, E], MMDT, tag="bvr")
            nc.sync.dma_start(out=bv_row,
                              in_=btv_r.rearrange("(one f) -> one f", one=1))
            ones_f = res.tile([1, 128], F32, tag="ones_f")
            ones_r = res.tile([1, 128], MMDT, tag="ones_r")
            nc.vector.memset(ones_f, 1.0)
            nc.scalar.copy(ones_r, ones_f)
            onesc_f = res.tile([128, H], F32, tag="onesc")
            nc.vector.memset(onesc_f, 1.0)

            # ---- long-lived activations ----
            KT = [res.tile([128, L], MMDT, tag=f"KT{j}", name=f"KT{j}")
                  for j in range(N_E)]
            # v_aug: head h occupies cols h*65..h*65+64, col h*65+64 == 1.0
            Vt = [res.tile([128, H * (D + 1)], MMDT, tag=f"V{lt}", name=f"V{lt}")
                  for lt in range(N_LT)]

            with tc.tile_pool(name="mn", bufs=1) as mn, \
                 tc.tile_pool(name="io", bufs=2) as io, \
                 tc.tile_pool(name="attp", bufs=2) as attp, \
                 tc.tile_pool(name="stp", bufs=3) as stp, \
                 tc.tile_pool(name="dnp", bufs=2) as dnp, \
                 tc.tile_pool(name="drp", bufs=2, space="DRAM") as drp:
                # weight loads, ordered so Wq (QT(0)) and Wtk (KT) land first
                Wq_sb = [mn.tile([128, E], MMDT, tag=f"wq{e}", name=f"wq{e}")
                         for e in range(N_E)]
                for e in range(N_E):
                    nc.sync.dma_start(out=Wq_sb[e], in_=Wq[e * 128:(e + 1) * 128, :])

                def emit_qin(c):
                    scs = slice(c * SC, (c + 1) * SC)
                    qT_in = [io.tile([128, SC], MMDT, tag=f"qin{e}",
                                     name=f"qin{e}_{c}")
                             for e in range(N_E)]
                    for e in range(N_E):
                        nc.sync.dma_start(out=qT_in[e],
                                          in_=queryT[e * 128:(e + 1) * 128, scs])
                    return qT_in

                def make_qt_tiles(c):
                    return [io.tile([128, SC], MMDT, tag=f"QT{j}",
                                    name=f"QT{j}_{c}")
                            for j in range(N_E)]

                def emit_qt_group(qT_in, QT, j):
                    """One QT j-group: 8 matmuls + scalar cast-with-bias."""
                    ps = psA.tile([128, SC], F32, tag="qacc")
                    for e in range(N_E):
                        nc.tensor.matmul(
                            ps, lhsT=Wq_sb[e][:, j * 128:(j + 1) * 128],
                            rhs=qT_in[e],
                            start=(e == 0), stop=(e == N_E - 1),
                        )
                    nc.scalar.activation(QT[j], ps, Ident,
                                         bias=bq_sb[:, j:j + 1])

                # QT(0) first: only needs Wq + qin(0), overlaps the K/V DMA
                qin_c = emit_qin(0)
                QT_c = make_qt_tiles(0)
                for j in range(N_E):
                    emit_qt_group(qin_c, QT_c, j)

                # ---- K/V prologue (from host-folded weights) ----
                with tc.tile_pool(name="pro", bufs=1) as pro, \
                     tc.tile_pool(name="psP", bufs=2, space="PSUM") as psP:
                    g_in = [pro.tile([128, L], MMDT, tag=f"gin{t}",
                                     name=f"gin{t}")
                            for t in range(N_TE)]
                    for t in range(N_TE):
                        nc.sync.dma_start(out=g_in[t],
                                          in_=guideT[t * 128:(t + 1) * 128, :])
                    Wtk_sb = [pro.tile([128, E], MMDT, tag=f"wtk{t}",
                                       name=f"wtk{t}")
                              for t in range(N_TE)]
                    for t in range(N_TE):
                        nc.sync.dma_start(out=Wtk_sb[t],
                                          in_=Wtk[t * 128:(t + 1) * 128, :])
                    Wtv_sb = [pro.tile([128, E], MMDT, tag=f"wtv{t}",
                                       name=f"wtv{t}")
                              for t in range(N_TE)]
                    for t in range(N_TE):
                        nc.sync.dma_start(out=Wtv_sb[t],
                                          in_=Wtv[t * 128:(t + 1) * 128, :])

                    # KT = Wtk^T @ guideT + btk
                    for j in range(N_E):
                        ps = psP.tile([128, SC], F32, tag="acc")
                        for t in range(N_TE):
                            nc.tensor.matmul(
                                ps, lhsT=Wtk_sb[t][:, j * 128:(j + 1) * 128],
                                rhs=g_in[t],
                                start=(t == 0), stop=(t == N_TE - 1),
                            )
                        nc.vector.tensor_scalar_add(KT[j], ps,
                                                    btk_sb[:, j:j + 1])

                    # V = guideT^T @ Wtv + btv, written into v_aug layout
                    for lt in range(N_LT):
                        va = Vt[lt].rearrange("p (h c) -> p h c", c=D + 1)
                        for half in range(2):
                            ps = psP.tile([128, SC], F32, tag="acc")
                            for t in range(N_TE):
                                nc.tensor.matmul(
                                    ps,
                                    lhsT=g_in[t][:, lt * 128:(lt + 1) * 128],
                                    rhs=Wtv_sb[t][:, half * SC:(half + 1) * SC],
                                    start=(t == 0), stop=False,
                                )
                            nc.tensor.matmul(
                                ps, lhsT=ones_r,
                                rhs=bv_row[:, half * SC:(half + 1) * SC],
                                start=False, stop=True,
                            )
                            nc.vector.tensor_copy(
                                va[:, half * 8:(half + 1) * 8, 0:D],
                                ps.rearrange("p (h c) -> p h c", c=D),
                            )
                        nc.vector.tensor_copy(
                            va[:, :, D:D + 1],
                            onesc_f.rearrange("p (h c) -> p h c", c=1),
                        )

                Wo_sb = [mn.tile([128, E], MMDT, tag=f"wo{e}", name=f"wo{e}")
                         for e in range(N_E)]
                for e in range(N_E):
                    nc.sync.dma_start(out=Wo_sb[e], in_=Wo[e * 128:(e + 1) * 128, :])

                psS_cm = tc.tile_pool(name="psS", bufs=2, space="PSUM")
                psS = psS_cm.__enter__()

                def emit_wo_group(c, OT, g):
                    """One Wo output group (st, half): 8 matmuls + bias/cast
                    + DMA out."""
                    st, half = divmod(g, 2)
                    sts = slice(st * 128, (st + 1) * 128)
                    ps = psC.tile([128, SC], F32, tag="wacc")
                    for j in range(N_E):
                        nc.tensor.matmul(
                            ps, lhsT=OT[j][:, sts],
                            rhs=Wo_sb[j][:, half * SC:(half + 1) * SC],
                            start=(j == 0), stop=(j == N_E - 1),
                        )
                    ob = stp.tile([128, SC], F32, tag="ob")
                    nc.vector.tensor_add(
                        ob, ps, bo_bc[:, half * SC:(half + 1) * SC])
                    nc.sync.dma_start(
                        out=out[c * SC + st * 128: c * SC + (st + 1) * 128,
                                half * SC:(half + 1) * SC],
                        in_=ob,
                    )

                def emit_hp(c, QT, OT, dns, denom, hp):
                    """scores + wide exp + PV for one head pair; raw PV
                    written to OT, denom rows staged into dns 32-aligned
                    slots and immediately DMA-gathered into `denom`
                    (normalization happens next chunk)."""
                    att = [attp.tile([128, 2 * SC], MMDT, tag=f"at{lt}",
                                     name=f"at{lt}_{c}_{hp}")
                           for lt in range(N_LT)]
                    for lt in range(N_LT):
                        lts = slice(lt * 128, (lt + 1) * 128)
                        ps = psS.tile([128, 2 * SC], F32, tag="scw")
                        for u in range(2):
                            rows = slice(u * 64, (u + 1) * 64)
                            nc.tensor.matmul(
                                ps[:, u * SC:(u + 1) * SC],
                                lhsT=KT[hp][rows, lts],
                                rhs=QT[hp][rows, :],
                                start=True, stop=True,
                            )
                        nc.scalar.activation(
                            att[lt], ps, Exp,
                            bias=mb_sb[:, lt:lt + 1], scale=SCALE,
                        )
                    for u in range(2):
                        h = 2 * hp + u
                        rows = slice(u * 64, (u + 1) * 64)
                        pv = psV.tile([D + 1, SC], F32, tag="pv",
                                      name=f"pv{h}_{c}")
                        for lt in range(N_LT):
                            nc.tensor.matmul(
                                pv,
                                lhsT=Vt[lt][:, h * (D + 1):(h + 1) * (D + 1)],
                                rhs=att[lt][:, u * SC:(u + 1) * SC],
                                start=(lt == 0), stop=(lt == N_LT - 1),
                            )
                        # raw (unnormalized) PV -> OT, denom row -> dns slot
                        nc.vector.tensor_copy(OT[hp][rows, :], pv[0:D, :])
                        sl = dns[32 * (h // 4):32 * (h // 4) + 1,
                                 (h % 4) * SC:(h % 4 + 1) * SC]
                        nc.vector.tensor_copy(sl, pv[D:D + 1, :])
                        nc.sync.dma_start(out=denom[h:h + 1, :], in_=sl)

                def emit_recip(c, denom, recips=None, rc_d=None,
                               n=H, bounce_from=0):
                    """Reciprocal over gathered denom rows [0,n) (engine
                    partition bases must be 32-aligned, so slices always
                    start at 0), cast to bf16 and bounced to DRAM for the
                    stride-0 partition broadcasts (the DMA slice has no
                    base restriction). bf16 recips keep the scale muls in
                    the DVE 2-byte fast path."""
                    if recips is None:
                        recips = dnp.tile([H, SC], F32, tag="rc",
                                          name=f"rc_{c}")
                        rc_d = drp.tile([H, SC], MMDT, tag="rcd",
                                        name=f"rcd_{c}")
                    rc_bf = dnp.tile([H, SC], MMDT, tag="rcb",
                                     name=f"rcb_{c}_{n}")
                    nc.vector.reciprocal_approx_fast(recips[0:n, :],
                                                     denom[0:n, :])
                    nc.vector.tensor_copy(rc_bf[0:n, :], recips[0:n, :])
                    nc.gpsimd.dma_start(out=rc_d[bounce_from:n, :],
                                        in_=rc_bf[bounce_from:n, :])
                    return recips, rc_d

                BCAST_ENGINES = None  # bound lazily (nc engines)

                def emit_bcast(rc_d, h):
                    """Stride-0 partition broadcast of one recip row from
                    DRAM, spread across four engine DMA queues."""
                    bc = stp.tile([128, SC], MMDT, tag=f"bc{h % 4}",
                                  name=f"bc{h}")
                    rsl = rc_d[h:h + 1, :]
                    # same queue as the rc_d bounce write, so the reads
                    # can't race it
                    eng = nc.gpsimd
                    eng.dma_start(
                        out=bc,
                        in_=bass.AP(tensor=rsl.tensor, offset=rsl.offset,
                                    ap=[[0, 128], [1, SC]]),
                    )
                    return bc

                def emit_scale_head(OT, bc, h):
                    u = h % 2
                    rows = slice(u * 64, (u + 1) * 64)
                    nc.vector.tensor_mul(OT[h // 2][rows, :],
                                         OT[h // 2][rows, :], bc[rows, :])

                # ---- steady-state pipeline over s-chunks ----
                # per hp slot of chunk c:
                #   hp 0: all 16 broadcasts of chunk c-1's recips
                #   hp 0-2: scale muls of chunk c-1 (6/6/4)
                #   hp 0-3: QT(c+1) groups 4-7 (emitted one chunk late so
                #           the last chunk also has QT filler)
                #   hp 3-7: Wo groups of chunk c-1 (2,2,2,1,1) -- after ALL
                #           muls; emission order is semantic order for Tile
                #   hp 4-7: QT(c+1) groups 0-3
                # after hp 7: reciprocal + DRAM bounce for chunk c itself.
                # Last chunk: recip for heads 0-11 fires before hp 6 and
                # their normalize overlaps hp 6-7; only heads 12-15 and the
                # Wo projection remain for the tail.
                WO_AT = {3: [0, 1], 4: [2, 3], 5: [4, 5], 6: [6], 7: [7]}
                qin_next = emit_qin(1)
                QT_next = make_qt_tiles(1)
                qt_late = None       # (qin, QT) groups 4-7, one chunk late
                prev = None          # (c, OT, rc_d) awaiting normalize + Wo
                for c in range(N_SC):
                    last = c == N_SC - 1
                    OT_c = [io.tile([128, SC], MMDT, tag=f"OT{j}",
                                    name=f"OT{j}_{c}")
                            for j in range(N_E)]
                    dns_c = dnp.tile([97, 4 * SC], F32, tag="dns",
                                     name=f"dns_{c}")
                    dn_c = dnp.tile([H, SC], F32, tag="dn", name=f"dn_{c}")
                    bcs = []
                    rcs_c, rcd_c = None, None
                    for hp in range(HP):
                        if prev is not None:
                            c_p, OT_p, rc_p = prev
                            if hp == 0:
                                bcs = [emit_bcast(rc_p, h) for h in range(H)]
                            for h in range(6 * hp, min(6 * hp + 6, H)):
                                emit_scale_head(OT_p, bcs[h], h)
                        emit_hp(c, QT_c, OT_c, dns_c, dn_c, hp)
                        if last and hp == 6:
                            # early reciprocal for heads 0-11 (pairs 0-5)
                            rcs_c, rcd_c = emit_recip(c, dn_c, n=12)
                            bcs_c = [emit_bcast(rcd_c, h) for h in range(12)]
                            for h in range(12):
                                emit_scale_head(OT_c, bcs_c[h], h)
                        if qt_late is not None and hp < 4:
                            emit_qt_group(qt_late[0], qt_late[1], 4 + hp)
                        if QT_next is not None and hp >= 4:
                            emit_qt_group(qin_next, QT_next, hp - 4)
                        if prev is not None:
                            for g in WO_AT.get(hp, ()):
                                emit_wo_group(c_p, OT_p, g)
                    if last:
                        _, rcd_c = emit_recip(c, dn_c, rcs_c, rcd_c,
                                              n=H, bounce_from=12)
                    else:
                        _, rcd_c = emit_recip(c, dn_c)
                    prev = (c, OT_c, rcd_c)
                    qt_late = (qin_next, QT_next) if QT_next is not None \
                        else None
                    QT_c = QT_next
                    if c + 2 < N_SC:
                        qin_next = emit_qin(c + 2)
                        QT_next = make_qt_tiles(c + 2)
                    else:
                        qin_next, QT_next = None, None
                # tail: normalize heads 12-15 + project the last chunk
                c_p, OT_p, rc_p = prev
                bcs = [emit_bcast(rc_p, h) for h in range(12, H)]
                for i, h in enumerate(range(12, H)):
                    emit_scale_head(OT_p, bcs[i], h)
                for g in range(8):
                    emit_wo_group(c_p, OT_p, g)
                psS_cm.__exit__(None, None, None)

    if not nc.is_finalized():
        nc.finalize()
    return nc


def kernel(query, guide_vector, attention_mask, Wt, bt, Wq, bq, Wkv, bkv, Wo, bo):
    global _CACHED_NC
    query = np.asarray(query, dtype=np.float32)
    guide_vector = np.asarray(guide_vector, dtype=np.float32)
    attention_mask = np.asarray(attention_mask)
    Wt_f = np.asarray(Wt, dtype=np.float32)
    bt_f = np.asarray(bt, dtype=np.float32)
    Wkv_f = np.asarray(Wkv, dtype=np.float32)
    bkv_f = np.asarray(bkv, dtype=np.float32)
    bq = np.asarray(bq, dtype=np.float32)
    bo = np.asarray(bo, dtype=np.float32)
    # host-fold the text projection into the K/V projections (exact algebra)
    Wtk_m = (Wt_f @ Wkv_f[:, :E]).astype(NPDT)
    Wtv_m = (Wt_f @ Wkv_f[:, E:]).astype(NPDT)
    btk_m = bt_f @ Wkv_f[:, :E] + bkv_f[:E]
    btv_m = (bt_f @ Wkv_f[:, E:] + bkv_f[E:]).astype(NPDT)
    Wq_m = np.asarray(Wq, dtype=np.float32).astype(NPDT)
    Wo_m = np.asarray(Wo, dtype=np.float32).astype(NPDT)

    if _CACHED_NC is None:
        _CACHED_NC = build_nc()
    nc = _CACHED_NC

    mb = np.where(attention_mask == 0, np.float32(-1e9), np.float32(0.0))
    in_maps = []
    for b in range(B):
        in_maps.append({
            "queryT": np.ascontiguousarray(query[b].T).astype(NPDT),
            "guideT": np.ascontiguousarray(guide_vector[b].T).astype(NPDT),
            "Wtk": Wtk_m, "Wtv": Wtv_m, "Wq": Wq_m, "Wo": Wo_m,
            "bq": bq, "btk": btk_m, "btv_r": btv_m,
            "bo": bo, "mbias": mb[b].astype(np.float32),
        })
    res = run_bass_kernel_spmd(nc, in_maps, list(range(B)), trace=TRACE)
    if TRACE:
        kernel.last_exec_time_ns = res.exec_time_ns
        kernel.last_results = res
    return np.stack([res.results[b]["out"] for b in range(B)])
